# revision 1
# baseline (speedup 1.0000x reference)
"""Bass/Tile SPMD kernel for nn_GATModel: GAT(2-layer) + BiLSTM + bilinear.

8 cores: core c -> (unit u = c%4, half = c//4).
units: 0=lstm1_fwd(p) 1=lstm1_bwd(p) 2=lstm2_fwd(h) 3=lstm2_bwd(h)
Each core: 16 samples of its graph, full GAT, xproj for its unit,
512-step recurrence (leading no-op pad for bwd), AllGather(4), bilinear.

Layout notes:
- All matmuls are f32r x f32r (walrus requires matching dtypes when either
  operand is f32/f32r); f32r ops emit no standalone InstLdweights, which
  cuts the instruction count heavily.
- GAT runs in two phases to fit the f32r weights in SBUF: phase A (layer 1,
  w1e resident) spills h1T per sample to DRAM; phase B (layer 2 + xproj,
  w2e/wihb resident) reloads it.
- LSTM gates use one 3-bank PSUM tile per step with batched activations
  (gate order repacked host-side to [i|f|o|g]); xr is staged in 8-step DMA
  blocks.
- Adjacency masks / dep one-hots are precomputed host-side.
"""
import numpy as np
import ml_dtypes
import concourse.bass as bass
import concourse.mybir as mybir
from concourse import bacc
from concourse.tile import TileContext

F32 = mybir.dt.float32
F32R = mybir.dt.float32r
BF16 = mybir.dt.bfloat16
AF = mybir.ActivationFunctionType
AL = mybir.AluOpType
AX = mybir.AxisListType

PROFILE_NO_CC = False  # skip collective (TimelineSim profiling only)

L, S, H, HH, DEP, NL = 256, 255, 768, 384, 81, 3
N = L + S          # 511
NP = 512           # padded seq
KH = H // 128      # 6 chunks of feature dim
K2 = 2 * KH        # 12 chunks of 2H
G4 = 4 * HH        # 1536 gate width
XBLK = 8           # xr staging block (steps per DMA)


def declare_tensors(nc, nsamp=16):
    I = lambda name, shape, dt: nc.dram_tensor(name, shape, dt, kind="ExternalInput")
    T = dict(
        xT=I("xT", [nsamp // 2, 128, KH, L], F32R),
        msk=I("msk", [nsamp // 2, 128, 1024], BF16),
        ehm=I("ehm", [nsamp // 2, DEP, S], F32R),
        identity=I("identity", [128, 128], F32),
        ztab_in=I("ztab_in", [2, DEP, 770], F32R),
        W1=I("W1", [2, 128, KH, 770], F32R),
        W2=I("W2", [128, K2, 770], F32R),
        Iw=I("Iw", [128, 2, NP], F32R),
        Id=I("Id", [128, 2, NP], F32R),
        Jw=I("Jw", [128, 2, NP], F32R),
        Jd=I("Jd", [128, 2, NP], F32R),
        ones=I("ones", [1, NP], F32R),
        Wihb=I("Wihb", [128, KH + 1, G4], F32R),
        Whh=I("Whh", [128, 3, G4], F32R),
        flags=I("flags", [128, 2], F32),
        bilW=I("bilW", [NL, 128, KH, H], F32),
        bilb=I("bilb", [nsamp, NL], F32),
        out=nc.dram_tensor("out", [nsamp, NL], F32, kind="ExternalOutput"),
        biasx=I("biasx", [NP, G4], F32R),
        xproj=nc.dram_tensor("xproj", [NP, nsamp + 1, G4], F32R),
        h1spill=nc.dram_tensor("h1spill", [nsamp // 2, 128, K2, N], F32R),
        z2cc_in=nc.dram_tensor("z2cc_in", [nsamp // 2, 128, 4096], F32R),
        z2cc_out=nc.dram_tensor("z2cc_out", [nsamp, 128, 4096], F32R),
        cc_in=nc.dram_tensor("cc_in", [3 * 128, nsamp], F32),
        cc_out=nc.dram_tensor("cc_out", [12 * 128, nsamp], F32),
    )
    return T


def build_nc(nsamp=16, nstep=NP, debug=()):
    nc = bacc.Bacc()
    T = declare_tensors(nc, nsamp)
    with TileContext(nc) as tc:
        _emit(nc, tc, T, nsamp, nstep, {})
    nc.finalize()
    return nc


def _emit(nc, tc, T, nsamp, nstep, dbg):
    xT, msk_d, ehm, identity, ztab_in, W1, W2 = (
        T["xT"], T["msk"], T["ehm"], T["identity"], T["ztab_in"], T["W1"],
        T["W2"])
    Iw, Id, Jw, Jd, ones, Wihb, Whh, flags, bilW, bilb = (
        T["Iw"], T["Id"], T["Jw"], T["Jd"], T["ones"], T["Wihb"], T["Whh"],
        T["flags"], T["bilW"], T["bilb"])
    out, xproj, cc_in, cc_out = T["out"], T["xproj"], T["cc_in"], T["cc_out"]
    biasx = T["biasx"]
    h1spill, z2cc_in, z2cc_out = T["h1spill"], T["z2cc_in"], T["z2cc_out"]

    def ts_(eng, o, i, s1, s2, o0, o1=None):
        if o1 is None:
            return eng.tensor_scalar(o, i, s1, s2, op0=o0)
        return eng.tensor_scalar(o, i, s1, s2, op0=o0, op1=o1)

    with tc.tile_pool(name="wper", bufs=1) as wper, \
         tc.tile_pool(name="glob", bufs=1) as glob:
        flg = wper.tile([128, 2], F32); nc.sync.dma_start(flg[:], flags[:])
        ident = glob.tile([128, 128], F32)
        nc.sync.dma_start(ident[:], identity[:])
        i17 = glob.tile([17, 16], F32R)
        on17 = glob.tile([17, 16], F32)
        nc.vector.memset(on17[:], 1.0)
        nc.vector.tensor_copy(i17[:], on17[:])
        nc.vector.tensor_copy(i17[0:16, :], ident[0:16, 0:16])
        ztab = []

        # ---------- shared attention helpers ----------
        scn = [0]

        def attn_block(attn, SB, er_cols, el_bc, neg_tiles, rows_l, src_n, alpk, guard):
            # guard=True: rows may be fully masked (word dst rows);
            # dep rows always have >=2 neighbors -> no guard.
            als = []
            for i, rows in enumerate(rows_l):
                xb = attn.tile([128, 256], F32, tag="ax")
                nc.vector.scalar_tensor_tensor(
                    xb[0:rows, 0:src_n], el_bc[0:rows, 0:src_n], er_cols[i],
                    neg_tiles[i][0:rows, 0:src_n], op0=AL.add, op1=AL.add)
                nc.vector.scalar_tensor_tensor(
                    xb[0:rows, 0:src_n], xb[0:rows, 0:src_n], 0.2,
                    xb[0:rows, 0:src_n], op0=AL.mult, op1=AL.max)
                cb = 1280 + 8 * ((scn[0]) % 8); scn[0] += 1
                nmx = SB[0:rows, cb:cb + 1]
                nc.vector.tensor_reduce(nmx, xb[0:rows, 0:src_n], AX.X, AL.max, negate=True)
                p = attn.tile([128, 256], F32, tag="ap")
                ssum = SB[0:rows, cb + 1:cb + 2]
                nc.scalar.activation(p[0:rows, 0:src_n], xb[0:rows, 0:src_n], AF.Exp,
                                     bias=nmx, scale=1.0, accum_out=ssum)
                r = SB[0:rows, cb + 2:cb + 3]
                nc.vector.reciprocal(r, ssum)
                if guard:
                    rg = SB[0:rows, cb + 4:cb + 5]
                    nc.vector.scalar_tensor_tensor(rg, nmx, 1e7, r,
                                                   op0=AL.is_lt, op1=AL.mult)
                else:
                    rg = r
                al = alpk[:, i, :]
                ts_(nc.vector, al[0:rows, 0:src_n], p[0:rows, 0:src_n], rg, None, AL.mult)
                als.append(al)
            return als

        def alT_mm(palt, feat, al_tiles, dst_rows_l, src_n, place, width, tag):
            src_tiles = []
            n_src_t = (src_n + 127) // 128
            for mi in range(n_src_t):
                mw = min(128, src_n - 128 * mi)
                ps = palt.tile([128, width], F32, tag=tag)
                for ki, dr in enumerate(dst_rows_l):
                    nc.tensor.matmul(ps[0:mw, :], al_tiles[ki][0:dr, 128 * mi:128 * mi + mw],
                                     place[0:dr, ki, :], start=(ki == 0), stop=(ki == len(dst_rows_l) - 1))
                sb = feat.tile([128, width], F32R, tag=tag + "s")
                nc.scalar.copy(sb[0:mw, :], ps[0:mw, :])
                src_tiles.append(sb)
            return src_tiles

        def vrow_transpose(pvec, SB, col_tiles, widths, dstap):
            w = sum(widths)
            rps = pvec.tile([1, 512], F32, tag="vrow")
            off = 0
            for ct, cw in zip(col_tiles, widths):
                nc.tensor.matmul(rps[:, off:off + cw], ct, ident[0:cw, 0:cw],
                                 is_transpose=True, start=True, stop=True)
                off += cw
            row = SB[0:1, 1024:1024 + w]
            nc.scalar.copy(row, rps[:, 0:w])
            nc.gpsimd.partition_broadcast(dstap[:, 0:w], row)

        # ======================= PHASE A: layer 1 (own half of samples) ===
        # core parity decides ownership via input data: fwd cores get local
        # samples 0..7, bwd cores 8..15 (host slices xT/msk/ehm accordingly).
        with tc.tile_pool(name="pA", bufs=1) as pA:
            w1e = []
            for h in range(2):
                t = pA.tile([128, KH, 770], F32R, tag=f"w1e{h}")
                nc.sync.dma_start(t[:], W1[h])
                w1e.append(t)
                zt = glob.tile([DEP, 770], F32R, tag=f"ztab{h}")
                nc.sync.dma_start(zt[:], ztab_in[h])
                ztab.append(zt)
            iwt = pA.tile([128, 2, NP], F32R); nc.sync.dma_start(iwt[:], Iw[:])
            idt = pA.tile([128, 2, NP], F32R); nc.sync.dma_start(idt[:], Id[:])

            with tc.tile_pool(name="samp", bufs=1) as samp, \
                 tc.tile_pool(name="attn", bufs=1) as attn, \
                 tc.tile_pool(name="feat", bufs=2) as feat, \
                 tc.tile_pool(name="fet1", bufs=1) as fet1, \
                 tc.tile_pool(name="big", bufs=2) as big, \
                 tc.tile_pool(name="pz", bufs=1, space="PSUM") as pz, \
                 tc.tile_pool(name="pagg", bufs=3, space="PSUM") as pagg, \
                 tc.tile_pool(name="palt", bufs=1, space="PSUM") as palt, \
                 tc.tile_pool(name="pvec", bufs=1, space="PSUM") as pvec:

                for s in range(nsamp // 2):
                    xts = samp.tile([128, KH, L], F32R, tag="xts")
                    nc.sync.dma_start(xts[:], xT[s])
                    mskt = samp.tile([128, 1024], BF16, tag="msk")
                    nc.sync.dma_start(mskt[:], msk_d[s])
                    negw = [mskt[:, 0:S], mskt[:, 256:256 + S]]
                    negd = [mskt[:, 512:768], mskt[:, 768:1024]]
                    eh = samp.tile([DEP, S], F32R, tag="eh")
                    nc.sync.dma_start(eh[:], ehm[s])
                    SB = samp.tile([128, 1344], F32, tag="scrb")
                    eldb, elwb = SB[:, 0:512], SB[:, 512:1024]

                    h1T = big.tile([128, K2, N], F32R, tag="h1T")
                    for h in range(2):
                        colt = fet1.tile([128, 32], F32, tag="cols")
                        zsb, erw_c, elw_c = [], [], []
                        for m in range(2):
                            zps = pz.tile([128, 1024], F32, tag="z")
                            for n0, nw in ((0, 512), (512, 258)):
                                for k in range(KH):
                                    nc.tensor.matmul(zps[:, n0:n0 + nw],
                                                     xts[:, k, 128 * m:128 * (m + 1)],
                                                     w1e[h][:, k, n0:n0 + nw],
                                                     start=(k == 0), stop=(k == KH - 1))
                            zb = feat.tile([128, H], F32R, tag="zw")
                            nc.scalar.copy(zb[:], zps[:, 0:H])
                            ec = colt[:, 16 * h + 2 * m:16 * h + 2 * m + 2]
                            nc.vector.tensor_copy(ec, zps[:, 768:770])
                            zsb.append(zb); elw_c.append(ec[:, 0:1]); erw_c.append(ec[:, 1:2])
                        zdsb, erd_c, eld_c = [], [], []
                        for m, rows in ((0, 128), (1, 127)):
                            zps = pz.tile([128, 1024], F32, tag="z")
                            for n0, nw in ((0, 512), (512, 258)):
                                nc.tensor.matmul(zps[0:rows, n0:n0 + nw],
                                                 eh[:, 128 * m:128 * m + rows],
                                                 ztab[h][:, n0:n0 + nw], start=True, stop=True)
                            zb = feat.tile([128, H], F32R, tag="zd")
                            nc.scalar.copy(zb[0:rows], zps[0:rows, 0:H])
                            ec = colt[:, 16 * h + 4 + 2 * m:16 * h + 4 + 2 * m + 2]
                            nc.vector.tensor_copy(ec[0:rows], zps[0:rows, 768:770])
                            zdsb.append(zb); eld_c.append(ec[0:rows, 0:1]); erd_c.append(ec[0:rows, 1:2])
                        vrow_transpose(pvec, SB, eld_c, [128, 127], eldb)
                        vrow_transpose(pvec, SB, elw_c, [128, 128], elwb)
                        alpW = attn.tile([128, 2, 256], F32R, tag="alw")
                        alpD = attn.tile([128, 2, 256], F32R, tag="ald")
                        alW = attn_block(attn, SB, erw_c, eldb, negw, [128, 128], S, alpW, True)
                        alD = attn_block(attn, SB, erd_c, elwb, negd, [128, 127], L, alpD, False)
                        aTW = alT_mm(palt, feat, alW, [128, 128], S, iwt, NP, "atw")
                        aTD = alT_mm(palt, feat, alD, [128, 127], L, idt, NP, "atd")
                        for wave in range(2):
                            pss = []
                            for mt in range(3 * wave, 3 * wave + 3):
                                ps = pagg.tile([128, NP], F32, tag="agg")
                                first = True
                                for ks, rows in ((0, 128), (1, 127)):
                                    nc.tensor.matmul(ps[:], zdsb[ks][0:rows, 128 * mt:128 * (mt + 1)],
                                                     aTW[ks][0:rows, :], start=first, stop=False)
                                    first = False
                                for ks in (0, 1):
                                    nc.tensor.matmul(ps[:], zsb[ks][:, 128 * mt:128 * (mt + 1)],
                                                     aTD[ks][:, :], start=False, stop=(ks == 1))
                                pss.append((mt, ps))
                            for mt, ps in pss:
                                ex = fet1.tile([128, N], F32, tag="elux")
                                nc.scalar.activation(ex[:], ps[:, 0:N], AF.Exp)
                                ts_(nc.vector, ex[:], ex[:], 1.0, -1.0, AL.min, AL.add)
                                nc.vector.scalar_tensor_tensor(
                                    h1T[:, KH * h + mt, :], ps[:, 0:N], 0.0, ex[:], op0=AL.max, op1=AL.add)
                    nc.sync.dma_start(h1spill[s], h1T[:])

        # ============ PHASE B1: layer-2 z + attention (own half) ============
        with tc.tile_pool(name="pB1", bufs=1) as pB1:
            w2e = pB1.tile([128, K2, 770], F32R)
            nc.sync.dma_start(w2e[:], W2[:])

            with tc.tile_pool(name="sampB", bufs=1) as sampB, \
                 tc.tile_pool(name="attnB", bufs=1) as attnB, \
                 tc.tile_pool(name="fetB", bufs=1) as fetB, \
                 tc.tile_pool(name="bigB", bufs=2) as bigB, \
                 tc.tile_pool(name="pzB", bufs=1, space="PSUM") as pzB, \
                 tc.tile_pool(name="pvecB", bufs=1, space="PSUM") as pvecB:

                for s in range(nsamp // 2):
                    h1T = bigB.tile([128, K2, N], F32R, tag="h1T")
                    nc.sync.dma_start(h1T[:], h1spill[s])
                    mskt = sampB.tile([128, 1024], BF16, tag="msk")
                    nc.sync.dma_start(mskt[:], msk_d[s])
                    negw = [mskt[:, 0:S], mskt[:, 256:256 + S]]
                    negd = [mskt[:, 512:768], mskt[:, 768:1024]]
                    SB = sampB.tile([128, 1344], F32, tag="scrb")
                    eldb, elwb = SB[:, 0:512], SB[:, 512:1024]

                    colt2 = fetB.tile([128, 16], F32, tag="cols2")
                    z2all = fetB.tile([128, 4, H], F32R, tag="z2all")
                    z2sb, er2_c, el2_c = [], [], []
                    for m, rows in ((0, 128), (1, 128), (2, 128), (3, 127)):
                        zps = pzB.tile([128, 1024], F32, tag="z")
                        for n0, nw in ((0, 512), (512, 258)):
                            for k in range(K2):
                                nc.tensor.matmul(zps[0:rows, n0:n0 + nw],
                                                 h1T[:, k, 128 * m:128 * m + rows],
                                                 w2e[:, k, n0:n0 + nw],
                                                 start=(k == 0), stop=(k == K2 - 1))
                        zb = z2all[:, m, :]
                        nc.scalar.copy(zb[0:rows], zps[0:rows, 0:H])
                        ec = colt2[:, 2 * m:2 * m + 2]
                        nc.vector.tensor_copy(ec[0:rows], zps[0:rows, 768:770])
                        z2sb.append(zb); el2_c.append(ec[0:rows, 0:1]); er2_c.append(ec[0:rows, 1:2])
                    vrow_transpose(pvecB, SB, el2_c[2:], [128, 127], eldb)
                    vrow_transpose(pvecB, SB, el2_c[:2], [128, 128], elwb)
                    alp2 = attnB.tile([128, 4, 256], F32R, tag="alp2")
                    attn_block(attnB, SB, er2_c[:2], eldb, negw, [128, 128], S, alp2[:, 0:2, :], True)
                    attn_block(attnB, SB, er2_c[2:], elwb, negd, [128, 127], L, alp2[:, 2:4, :], False)
                    nc.sync.dma_start(z2cc_in[s, :, 0:4 * H],
                                      z2all[:].rearrange("p c f -> p (c f)"))
                    nc.sync.dma_start(z2cc_in[s, :, 4 * H:4096],
                                      alp2[:].rearrange("p c f -> p (c f)"))

        # pairwise exchange of z2 + attention (the only cross-core dedup point)
        nc.gpsimd.collective_compute(
            "AllGather", AL.bypass,
            replica_groups=[[0, 1], [2, 3], [4, 5], [6, 7]],
            ins=[z2cc_in.ap().opt()], outs=[z2cc_out.ap().opt()])

        # ============ PHASE B2: alT + aggregation + xproj (all samples) =====
        with tc.tile_pool(name="pB2", bufs=1) as pB2:
            wihb = pB2.tile([128, KH + 1, G4], F32R)
            nc.sync.dma_start(wihb[:], Wihb[:])
            jwt = pB2.tile([128, 2, NP], F32R); nc.sync.dma_start(jwt[:], Jw[:])
            jdt = pB2.tile([128, 2, NP], F32R); nc.sync.dma_start(jdt[:], Jd[:])
            # bias rides in xproj slot nsamp (pad step zeroed host-side)
            nc.sync.dma_start(xproj[:, nsamp, :].unsqueeze(1), biasx[:].unsqueeze(1))

            with tc.tile_pool(name="sampC", bufs=2) as sampC, \
                 tc.tile_pool(name="featC", bufs=2) as featC, \
                 tc.tile_pool(name="bigC", bufs=1) as bigC, \
                 tc.tile_pool(name="paggC", bufs=3, space="PSUM") as paggC, \
                 tc.tile_pool(name="paltC", bufs=1, space="PSUM") as paltC:

                for s in range(nsamp):
                    pk = sampC.tile([128, 4096], F32R, tag="pk")
                    nc.sync.dma_start(pk[:], z2cc_out[s])
                    z2sb = [pk[:, H * m:H * (m + 1)] for m in range(4)]
                    alW2 = [pk[:, 3072:3328], pk[:, 3328:3584]]
                    alD2 = [pk[:, 3584:3840], pk[:, 3840:4096]]
                    aTW2 = alT_mm(paltC, featC, alW2, [128, 128], S, jwt, NP, "atw")
                    aTD2 = alT_mm(paltC, featC, alD2, [128, 127], L, jdt, NP, "atd")
                    gatT = bigC.tile([128, KH, NP], F32R, tag="gatT")
                    for wave in range(2):
                        pss = []
                        for mt in range(3 * wave, 3 * wave + 3):
                            ps = paggC.tile([128, NP], F32, tag="agg")
                            first = True
                            for ks, rows in ((0, 128), (1, 127)):
                                nc.tensor.matmul(ps[:], z2sb[2 + ks][0:rows, 128 * mt:128 * (mt + 1)],
                                                 aTW2[ks][0:rows, :], start=first, stop=False)
                                first = False
                            for ks in (0, 1):
                                nc.tensor.matmul(ps[:], z2sb[ks][:, 128 * mt:128 * (mt + 1)],
                                                 aTD2[ks][:, :], start=False, stop=(ks == 1))
                            pss.append((mt, ps))
                        for mt, ps in pss:
                            nc.scalar.copy(gatT[:, mt, :], ps[:])

                    # ---- xproj ----
                    for m in range(4):
                        pss = []
                        for ni in range(3):
                            ps = paggC.tile([128, 512], F32, tag="agg")
                            for k in range(KH):
                                nc.tensor.matmul(ps[:], gatT[:, k, 128 * m:128 * (m + 1)],
                                                 wihb[:, k, 512 * ni:512 * (ni + 1)],
                                                 start=(k == 0), stop=(k == KH - 1))
                            pss.append((ni, ps))
                        xsb = bigC.tile([128, G4], F32R, tag="xsb")
                        for ni, ps in pss:
                            nc.scalar.copy(xsb[:, 512 * ni:512 * (ni + 1)], ps[:])
                        nc.sync.dma_start(xproj[128 * m:128 * (m + 1), s:s + 1, :], xsb[:].unsqueeze(1))

        # ================= recurrence =================
        # gates packed host-side as [i|f|o|g]: one sigmoid over [0:3HH],
        # one tanh over [3HH:4HH].
        with tc.tile_pool(name="rx", bufs=2) as rx, \
             tc.tile_pool(name="rst", bufs=2) as rst, \
             tc.tile_pool(name="rg", bufs=2) as rg, \
             tc.tile_pool(name="pg", bufs=2, space="PSUM") as pgp, \
             tc.tile_pool(name="ptr", bufs=2, space="PSUM") as ptr, \
             tc.tile_pool(name="rfin", bufs=1) as rfin:
            whhr = rfin.tile([128, 3, G4], F32R)
            nc.sync.dma_start(whhr[:], Whh[:])
            W3 = 3 * nsamp
            hT = rst.tile([128, W3], F32R, tag="hT")
            zed = rfin.tile([128, W3], F32)
            nc.vector.memset(zed[:], 0.0)
            nc.vector.tensor_copy(hT[:], zed[:])
            cst = rst.tile([nsamp, HH], F32, tag="c")
            nc.vector.memset(cst[:], 0.0)
            snapA = rfin.tile([128, W3], F32)
            snapB = rfin.tile([128, W3], F32)
            SH3 = 3 * HH
            for t in range(nstep):
                tb = t % XBLK
                if tb == 0:
                    xrb = rx.tile([nsamp + 1, XBLK, G4], F32R, tag="xr")
                    nc.sync.dma_start(
                        xrb[:], xproj[t:t + XBLK].rearrange("t s g -> s t g"))
                ps = pgp.tile([nsamp, G4], F32, tag="gates")
                for b in range(3):
                    sl = slice(512 * b, 512 * (b + 1))
                    nc.tensor.matmul(ps[:, sl], i17[0:nsamp + 1, 0:nsamp], xrb[:, tb, sl],
                                     start=True, stop=False)
                    for kc in range(3):
                        nc.tensor.matmul(ps[:, sl], hT[:, nsamp * kc:nsamp * (kc + 1)],
                                         whhr[:, kc, sl], start=False, stop=(kc == 2))
                sg = rg.tile([nsamp, SH3], F32, tag="sg")
                nc.scalar.activation(sg[:], ps[:, 0:SH3], AF.Sigmoid)
                tg = rg.tile([nsamp, HH], F32, tag="tg")
                nc.scalar.activation(tg[:], ps[:, SH3:G4], AF.Tanh)
                t1 = rg.tile([nsamp, HH], F32, tag="t1")
                nc.vector.tensor_mul(t1[:], sg[:, HH:2 * HH], cst[:])
                t2 = rg.tile([nsamp, HH], F32, tag="t2")
                nc.vector.tensor_mul(t2[:], sg[:, 0:HH], tg[:])
                cn = rst.tile([nsamp, HH], F32, tag="c")
                nc.vector.tensor_add(cn[:], t1[:], t2[:])
                th = rg.tile([nsamp, HH], F32, tag="th")
                nc.scalar.activation(th[:], cn[:], AF.Tanh)
                hh = rg.tile([nsamp, HH], F32, tag="hh")
                nc.vector.tensor_mul(hh[:], sg[:, 2 * HH:SH3], th[:])
                tps = ptr.tile([128, W3], F32, tag="tr")
                for kc in range(3):
                    nc.tensor.matmul(tps[:, nsamp * kc:nsamp * (kc + 1)],
                                     hh[:, 128 * kc:128 * (kc + 1)], ident[0:nsamp, 0:nsamp],
                                     is_transpose=True, start=True, stop=True)
                hTn = rst.tile([128, W3], F32R, tag="hT")
                nc.vector.tensor_copy(hTn[:], tps[:])
                hT = hTn
                cst = cn
                if t == nstep - 2:
                    nc.vector.tensor_copy(snapA[:], tps[:])
                if t == nstep - 1:
                    nc.vector.tensor_copy(snapB[:], tps[:])
            hsel = rfin.tile([128, W3], F32)
            ts_(nc.vector, hsel[:], snapA[:], flg[:, 0:1], None, AL.mult)
            nc.vector.scalar_tensor_tensor(hsel[:], snapB[:], flg[:, 1:2], hsel[:],
                                           op0=AL.mult, op1=AL.add)
            nc.sync.dma_start(cc_in[:].rearrange("(c p) f -> p c f", p=128),
                              hsel[:].rearrange("p (c f) -> p c f", c=3))

        # ================= collective + bilinear =================
        with tc.tile_pool(name="bil", bufs=1) as bil, \
             tc.tile_pool(name="pbil", bufs=1, space="PSUM") as pbil:
            if PROFILE_NO_CC:
                nc.sync.dma_start(cc_out[0:3 * 128, :], cc_in[:])
            else:
                nc.gpsimd.collective_compute(
                    "AllGather", AL.bypass,
                    replica_groups=[[0, 1, 2, 3], [4, 5, 6, 7]],
                    ins=[cc_in.ap().opt()], outs=[cc_out.ap().opt()])
            gath = bil.tile([128, 12, nsamp], F32)
            nc.sync.dma_start(gath[:], cc_out[:].rearrange("(c p) f -> p c f", p=128))
            pT = bil.tile([128, KH, nsamp], F32R)
            nc.vector.tensor_copy(pT[:], gath[:, 0:KH, :])
            hps = pbil.tile([nsamp, H], F32, tag="htr")
            for kc in range(KH):
                nc.tensor.matmul(hps[:, 128 * kc:128 * (kc + 1)],
                                 gath[:, KH + kc, :], ident[:],
                                 is_transpose=True, start=True, stop=True)
            hsb = bil.tile([nsamp, H], F32)
            nc.scalar.copy(hsb[:], hps[:])
            bwsb = bil.tile([128, KH, H], F32)
            bwr = bil.tile([128, KH, H], F32R)
            outc = bil.tile([nsamp, NL], F32)
            junk = bil.tile([nsamp, H], F32)
            for k in range(NL):
                nc.sync.dma_start(bwsb[:], bilW[k])
                for c in range(KH):
                    nc.vector.tensor_copy(bwr[:, c, :], bwsb[:, c, :])
                vps = pbil.tile([nsamp, H], F32, tag="v")
                for n0 in (0, 512):
                    nw = min(512, H - n0)
                    for kc in range(KH):
                        nc.tensor.matmul(vps[:, n0:n0 + nw], pT[:, kc, :],
                                         bwr[:, kc, n0:n0 + nw],
                                         start=(kc == 0), stop=(kc == KH - 1))
                nc.vector.scalar_tensor_tensor(junk[:], vps[:], 1.0, hsb[:],
                                               op0=AL.mult, op1=AL.mult,
                                               accum_out=outc[:, k:k + 1])
            bbt = bil.tile([nsamp, NL], F32)
            nc.sync.dma_start(bbt[:], bilb[:])
            outt = bil.tile([nsamp, NL], F32)
            nc.vector.tensor_add(outt[:], outc[:], bbt[:])
            nc.sync.dma_start(out[:], outt[:])


# ===================== host-side preparation =====================
def _chunkP(a):
    """[X*128, ...] -> [128, X, ...] with p inner: out[p, c, ...] = a[128c+p]"""
    x = a.reshape(a.shape[0] // 128, 128, *a.shape[1:])
    return np.swapaxes(x, 0, 1)

def _bf(a):
    return np.ascontiguousarray(a.astype(ml_dtypes.bfloat16))

def _f(a):
    return np.ascontiguousarray(np.asarray(a, np.float32))

# gate reorder: torch [i|f|g|o] -> kernel [i|f|o|g]
_GPERM = np.concatenate([np.arange(0, HH), np.arange(HH, 2 * HH),
                         np.arange(3 * HH, 4 * HH), np.arange(2 * HH, 3 * HH)])


def _build_masks(spans, nsamp):
    """spans [nsamp,S,3] int -> (msk [nsamp,128,1024] bf16, eh [nsamp,DEP,S] f32).

    msk cols: 0:255 negw(nodes 0..127) | 256:511 negw(nodes 128..255)
            | 512:768 negd(deps 0..127) | 768:1024 negd(deps 128..254)
    """
    sp = np.asarray(spans)
    w0, w1, lab = sp[:, :, 0], sp[:, :, 1], sp[:, :, 2]   # [nsamp, S]
    nodes = np.arange(256)
    adjW = ((w0[:, None, :] == nodes[None, :, None]) |
            (w1[:, None, :] == nodes[None, :, None]))     # [nsamp,256,S]
    negW = (adjW.astype(np.float32) - 1.0) * 1e9
    words = np.arange(256)
    adjD = ((w0[:, :, None] == words[None, None, :]) |
            (w1[:, :, None] == words[None, None, :]))     # [nsamp,S,256]
    negD = (adjD.astype(np.float32) - 1.0) * 1e9
    msk = np.full((nsamp, 128, 1024), -1e9, np.float32)
    msk[:, :, 0:S] = negW[:, 0:128]
    msk[:, :, 256:256 + S] = negW[:, 128:256]
    msk[:, :, 512:768] = negD[:, 0:128]
    msk[:, 0:127, 768:1024] = negD[:, 128:255]
    eh = (lab[:, None, :] == np.arange(DEP)[None, :, None]).astype(np.float32)
    return _bf(msk), _f(eh)


def make_in_maps(inp, nsamp=16):
    B = np.asarray(inp["prem_hidden_states"]).shape[0]
    emb = _f(inp["depend_emb"])
    in_maps = []
    onehot = lambda idx, w: np.eye(w, dtype=np.float32)[idx]  # rows
    # placement mats (constant)
    wid = np.arange(256)
    Iw_ = _f(_chunkP(onehot(wid, NP)))                # -> [128,2,NP]
    sid = np.arange(255)
    Id_rows = np.zeros((256, NP), np.float32); Id_rows[:255] = onehot(256 + sid, NP)
    Id_ = _f(_chunkP(Id_rows))
    ident_ = np.eye(128, dtype=np.float32)
    for c in range(8):
        unit, half = c % 4, c // 4
        g = "prem" if unit < 2 else "hypo"
        fwd = (unit % 2 == 0)
        lstm = "lstm1" if unit < 2 else "lstm2"
        dirn = "f" if fwd else "b"
        sl = slice(16 * half, 16 * half + nsamp)
        hid = _f(inp[f"{g}_hidden_states"])[sl]       # [nsamp, L, H]
        spans = np.asarray(inp[f"{g}_span"])[sl]      # [nsamp, S, 3]
        # pair dedup: fwd cores own local samples 0..7, bwd cores 8..15
        own = slice(0, nsamp // 2) if fwd else slice(nsamp // 2, nsamp)
        hid_o, spans_o = hid[own], spans[own]
        m = {}
        m["xT"] = _f(np.stack([_chunkP(hid_o[s].T) for s in range(nsamp // 2)]))
        m["msk"], m["ehm"] = _build_masks(spans_o, nsamp // 2)
        m["identity"] = ident_
        W1 = _f(inp[f"{g}_W1"])                       # [2,H,H]
        a1 = _f(inp[f"{g}_a1"])                       # [2, 2H]
        # extended weights: cols 768+lr hold W @ a_half (attention vectors)
        W1x = np.zeros((2, H, 770), np.float32)
        W1x[:, :, 0:H] = W1
        for h in range(2):
            for lr in range(2):
                W1x[h, :, 768 + lr] = W1[h] @ a1[h, lr * H:(lr + 1) * H]
        m["W1"] = _f(np.stack([_chunkP(W1x[h]) for h in range(2)]))
        m["ztab_in"] = _f(np.stack([emb @ W1x[h] for h in range(2)]))
        W2 = _f(inp[f"{g}_W2"])                       # [2H, H]
        a2 = _f(inp[f"{g}_a2"])                       # [2H]
        W2x = np.zeros((2 * H, 770), np.float32)
        W2x[:, 0:H] = W2
        for lr in range(2):
            W2x[:, 768 + lr] = W2 @ a2[lr * H:(lr + 1) * H]
        m["W2"] = _f(_chunkP(W2x))
        m["Iw"] = Iw_; m["Id"] = Id_
        if fwd:
            Jw_r = onehot(wid, NP)
            Jd_rows = np.zeros((256, NP), np.float32); Jd_rows[:255] = onehot(256 + sid, NP)
            ones_ = np.ones((1, NP), np.float32); ones_[0, N] = 0.0
        else:
            Jw_r = onehot(511 - wid, NP)
            Jd_rows = np.zeros((256, NP), np.float32); Jd_rows[:255] = onehot(255 - sid, NP)
            ones_ = np.ones((1, NP), np.float32); ones_[0, 0] = 0.0
        m["Jw"] = _f(_chunkP(Jw_r)); m["Jd"] = _f(_chunkP(Jd_rows))
        m["ones"] = _f(ones_)
        Wih = _f(inp[f"{lstm}_Wih_{dirn}"])           # [4HH, H]
        bb = _f(inp[f"{lstm}_b_{dirn}"])              # [4HH]
        Wihb_ = np.zeros((896, G4), np.float32)
        Wihb_[:H] = Wih.T[:, _GPERM]
        Wihb_[H] = bb[_GPERM]
        m["Wihb"] = _f(_chunkP(Wihb_))                # [128, 7, G4]
        biasx_ = np.tile(bb[_GPERM][None, :], (NP, 1)).astype(np.float32)
        biasx_[N if fwd else 0] = 0.0
        m["biasx"] = _f(biasx_)
        Whh_ = _f(inp[f"{lstm}_Whh_{dirn}"])          # [4HH, HH]
        m["Whh"] = _f(_chunkP(Whh_.T[:, _GPERM]))     # [128, 3, G4]
        fl = np.zeros((128, 2), np.float32)
        fl[:, 0] = 1.0 if fwd else 0.0
        fl[:, 1] = 0.0 if fwd else 1.0
        m["flags"] = fl
        bilW = _f(inp["bil_W"])                       # [3,H,H]
        m["bilW"] = _f(np.stack([_chunkP(bilW[k]) for k in range(NL)]))
        m["bilb"] = _f(np.broadcast_to(_f(inp["bil_b"])[None, :], (nsamp, NL)).copy())
        in_maps.append(m)
    return in_maps


# ===================== harness entry point =====================
_NC_CACHE = {}

def _get_nc(nsamp=16, nstep=NP):
    key = (nsamp, nstep)
    if key not in _NC_CACHE:
        _NC_CACHE[key] = build_nc(nsamp=nsamp, nstep=nstep)
    return _NC_CACHE[key]


def kernel(**inputs):
    """Full-input entry: shards across 8 NeuronCores, runs the Bass kernel,
    returns the full [32, 3] float32 output."""
    inputs = {k: np.asarray(v) for k, v in inputs.items()}
    nc = _get_nc()
    in_maps = make_in_maps(inputs, nsamp=16)
    from concourse import bass_utils
    res = bass_utils.run_bass_kernel_spmd(nc, in_maps, core_ids=list(range(8)))
    out = np.concatenate([res.results[0]["out"], res.results[4]["out"]], 0)
    return out.astype(np.float32)



# revision 5
# speedup vs baseline: 1.0453x; 1.0453x over previous
"""Bass/Tile SPMD kernel for nn_GATModel: GAT(2-layer) + BiLSTM + bilinear.

8 cores: core c -> (unit u = c%4, half = c//4).
units: 0=lstm1_fwd(p) 1=lstm1_bwd(p) 2=lstm2_fwd(h) 3=lstm2_bwd(h)
Each core: 16 samples of its graph, full GAT, xproj for its unit,
512-step recurrence (leading no-op pad for bwd), AllGather(4), bilinear.

Layout notes:
- All matmuls are f32r x f32r (walrus requires matching dtypes when either
  operand is f32/f32r); f32r ops emit no standalone InstLdweights, which
  cuts the instruction count heavily.
- GAT runs in two phases to fit the f32r weights in SBUF: phase A (layer 1,
  w1e resident) spills h1T per sample to DRAM; phase B (layer 2 + xproj,
  w2e/wihb resident) reloads it.
- LSTM gates use one 3-bank PSUM tile per step with batched activations
  (gate order repacked host-side to [i|f|o|g]); xr is staged in 8-step DMA
  blocks.
- Adjacency masks / dep one-hots are precomputed host-side.
"""
import numpy as np
import ml_dtypes
import concourse.bass as bass
import concourse.mybir as mybir
from concourse import bacc
from concourse.tile import TileContext

F32 = mybir.dt.float32
F32R = mybir.dt.float32r
BF16 = mybir.dt.bfloat16
AF = mybir.ActivationFunctionType
AL = mybir.AluOpType
AX = mybir.AxisListType

PROFILE_NO_CC = False  # skip collective (TimelineSim profiling only)

L, S, H, HH, DEP, NL = 256, 255, 768, 384, 81, 3
N = L + S          # 511
NP = 512           # padded seq
KH = H // 128      # 6 chunks of feature dim
K2 = 2 * KH        # 12 chunks of 2H
G4 = 4 * HH        # 1536 gate width
XBLK = 8           # xr staging block (steps per DMA)


def declare_tensors(nc, nsamp=16):
    I = lambda name, shape, dt: nc.dram_tensor(name, shape, dt, kind="ExternalInput")
    T = dict(
        xT=I("xT", [nsamp // 2, 128, KH, L], F32R),
        msk=I("msk", [nsamp // 2, 128, 1024], BF16),
        ehm=I("ehm", [nsamp // 2, DEP, S], F32R),
        identity=I("identity", [128, 128], F32),
        ztab_in=I("ztab_in", [2, DEP, 770], F32R),
        W1=I("W1", [2, 128, KH, 770], F32R),
        W2=I("W2", [128, K2, 770], F32R),
        Iw=I("Iw", [128, 2, NP], F32R),
        Id=I("Id", [128, 2, NP], F32R),
        Jw=I("Jw", [128, 2, NP], F32R),
        Jd=I("Jd", [128, 2, NP], F32R),
        ones=I("ones", [1, NP], F32R),
        Wihb=I("Wihb", [128, KH + 1, G4], F32R),
        Whh=I("Whh", [128, 3, G4], F32R),
        flags=I("flags", [128, 2], F32),
        bilW=I("bilW", [NL, 128, KH, H], F32),
        bilb=I("bilb", [nsamp, NL], F32),
        out=nc.dram_tensor("out", [nsamp, NL], F32, kind="ExternalOutput"),
        biasx=I("biasx", [NP, G4], F32R),
        xproj=nc.dram_tensor("xproj", [NP, nsamp + 1, G4], F32R),
        h1spill=nc.dram_tensor("h1spill", [nsamp // 2, 128, K2, N], F32R),
        z2cc_in=nc.dram_tensor("z2cc_in", [nsamp // 2, 128, 4096], F32R),
        z2cc_out=nc.dram_tensor("z2cc_out", [nsamp, 128, 4096], F32R),
        cc_in=nc.dram_tensor("cc_in", [3 * 128, nsamp], F32),
        cc_out=nc.dram_tensor("cc_out", [12 * 128, nsamp], F32),
    )
    return T


def build_nc(nsamp=16, nstep=NP, debug=()):
    nc = bacc.Bacc()
    T = declare_tensors(nc, nsamp)
    with TileContext(nc) as tc:
        _emit(nc, tc, T, nsamp, nstep, {})
    nc.finalize()
    return nc


def _emit(nc, tc, T, nsamp, nstep, dbg):
    xT, msk_d, ehm, identity, ztab_in, W1, W2 = (
        T["xT"], T["msk"], T["ehm"], T["identity"], T["ztab_in"], T["W1"],
        T["W2"])
    Iw, Id, Jw, Jd, ones, Wihb, Whh, flags, bilW, bilb = (
        T["Iw"], T["Id"], T["Jw"], T["Jd"], T["ones"], T["Wihb"], T["Whh"],
        T["flags"], T["bilW"], T["bilb"])
    out, xproj, cc_in, cc_out = T["out"], T["xproj"], T["cc_in"], T["cc_out"]
    biasx = T["biasx"]
    h1spill, z2cc_in, z2cc_out = T["h1spill"], T["z2cc_in"], T["z2cc_out"]

    def ts_(eng, o, i, s1, s2, o0, o1=None):
        if o1 is None:
            return eng.tensor_scalar(o, i, s1, s2, op0=o0)
        return eng.tensor_scalar(o, i, s1, s2, op0=o0, op1=o1)

    with tc.tile_pool(name="wper", bufs=1) as wper, \
         tc.tile_pool(name="glob", bufs=1) as glob:
        flg = wper.tile([128, 2], F32); nc.sync.dma_start(flg[:], flags[:])
        ident = glob.tile([128, 128], F32)
        nc.sync.dma_start(ident[:], identity[:])
        i17 = glob.tile([17, 16], F32R)
        on17 = glob.tile([17, 16], F32)
        nc.vector.memset(on17[:], 1.0)
        nc.vector.tensor_copy(i17[:], on17[:])
        nc.vector.tensor_copy(i17[0:16, :], ident[0:16, 0:16])
        ztab = []

        # ---------- shared attention helpers ----------
        scn = [0]

        def attn_block(attn, SB, er_cols, el_bc, neg_tiles, rows_l, src_n, alpk, guard):
            # guard=True: rows may be fully masked (word dst rows);
            # dep rows always have >=2 neighbors -> no guard.
            als = []
            for i, rows in enumerate(rows_l):
                xb = attn.tile([128, 256], F32, tag="ax")
                nc.vector.scalar_tensor_tensor(
                    xb[0:rows, 0:src_n], el_bc[0:rows, 0:src_n], er_cols[i],
                    neg_tiles[i][0:rows, 0:src_n], op0=AL.add, op1=AL.add)
                nc.vector.scalar_tensor_tensor(
                    xb[0:rows, 0:src_n], xb[0:rows, 0:src_n], 0.2,
                    xb[0:rows, 0:src_n], op0=AL.mult, op1=AL.max)
                cb = 1280 + 8 * ((scn[0]) % 8); scn[0] += 1
                nmx = SB[0:rows, cb:cb + 1]
                nc.vector.tensor_reduce(nmx, xb[0:rows, 0:src_n], AX.X, AL.max, negate=True)
                p = attn.tile([128, 256], F32, tag="ap")
                ssum = SB[0:rows, cb + 1:cb + 2]
                nc.scalar.activation(p[0:rows, 0:src_n], xb[0:rows, 0:src_n], AF.Exp,
                                     bias=nmx, scale=1.0, accum_out=ssum)
                r = SB[0:rows, cb + 2:cb + 3]
                nc.vector.reciprocal(r, ssum)
                if guard:
                    rg = SB[0:rows, cb + 4:cb + 5]
                    nc.vector.scalar_tensor_tensor(rg, nmx, 1e7, r,
                                                   op0=AL.is_lt, op1=AL.mult)
                else:
                    rg = r
                al = alpk[:, i, :]
                ts_(nc.vector, al[0:rows, 0:src_n], p[0:rows, 0:src_n], rg, None, AL.mult)
                als.append(al)
            return als

        def alT_mm(palt, feat, al_tiles, dst_rows_l, src_n, place, width, tag):
            src_tiles = []
            n_src_t = (src_n + 127) // 128
            for mi in range(n_src_t):
                mw = min(128, src_n - 128 * mi)
                ps = palt.tile([128, width], F32, tag=tag)
                for ki, dr in enumerate(dst_rows_l):
                    nc.tensor.matmul(ps[0:mw, :], al_tiles[ki][0:dr, 128 * mi:128 * mi + mw],
                                     place[0:dr, ki, :], start=(ki == 0), stop=(ki == len(dst_rows_l) - 1))
                sb = feat.tile([128, width], F32R, tag=tag + "s")
                nc.scalar.copy(sb[0:mw, :], ps[0:mw, :])
                src_tiles.append(sb)
            return src_tiles

        def vrow_transpose(pvec, SB, col_tiles, widths, dstap):
            w = sum(widths)
            rps = pvec.tile([1, 512], F32, tag="vrow")
            off = 0
            for ct, cw in zip(col_tiles, widths):
                nc.tensor.matmul(rps[:, off:off + cw], ct, ident[0:cw, 0:cw],
                                 is_transpose=True, start=True, stop=True)
                off += cw
            row = SB[0:1, 1024:1024 + w]
            nc.scalar.copy(row, rps[:, 0:w])
            nc.gpsimd.partition_broadcast(dstap[:, 0:w], row)

        # ======================= PHASE A: layer 1 (own half of samples) ===
        # core parity decides ownership via input data: fwd cores get local
        # samples 0..7, bwd cores 8..15 (host slices xT/msk/ehm accordingly).
        with tc.tile_pool(name="pA", bufs=1) as pA:
            w1e = []
            for h in range(2):
                t = pA.tile([128, KH, 770], F32R, tag=f"w1e{h}")
                nc.sync.dma_start(t[:], W1[h])
                w1e.append(t)
                zt = glob.tile([DEP, 770], F32R, tag=f"ztab{h}")
                nc.sync.dma_start(zt[:], ztab_in[h])
                ztab.append(zt)
            iwt = pA.tile([128, 2, NP], F32R); nc.sync.dma_start(iwt[:], Iw[:])
            idt = pA.tile([128, 2, NP], F32R); nc.sync.dma_start(idt[:], Id[:])

            with tc.tile_pool(name="samp", bufs=1) as samp, \
                 tc.tile_pool(name="attn", bufs=1) as attn, \
                 tc.tile_pool(name="feat", bufs=2) as feat, \
                 tc.tile_pool(name="fet1", bufs=1) as fet1, \
                 tc.tile_pool(name="big", bufs=2) as big, \
                 tc.tile_pool(name="pz", bufs=1, space="PSUM") as pz, \
                 tc.tile_pool(name="pagg", bufs=3, space="PSUM") as pagg, \
                 tc.tile_pool(name="palt", bufs=1, space="PSUM") as palt, \
                 tc.tile_pool(name="pvec", bufs=1, space="PSUM") as pvec:

                for s in range(nsamp // 2):
                    xts = samp.tile([128, KH, L], F32R, tag="xts")
                    nc.sync.dma_start(xts[:], xT[s])
                    mskt = samp.tile([128, 1024], BF16, tag="msk")
                    nc.sync.dma_start(mskt[:], msk_d[s])
                    negw = [mskt[:, 0:S], mskt[:, 256:256 + S]]
                    negd = [mskt[:, 512:768], mskt[:, 768:1024]]
                    eh = samp.tile([DEP, S], F32R, tag="eh")
                    nc.sync.dma_start(eh[:], ehm[s])
                    SB = samp.tile([128, 1344], F32, tag="scrb")
                    eldb, elwb = SB[:, 0:512], SB[:, 512:1024]

                    h1T = big.tile([128, K2, N], F32R, tag="h1T")
                    for h in range(2):
                        colt = fet1.tile([128, 32], F32, tag="cols")
                        zsb, erw_c, elw_c = [], [], []
                        for m in range(2):
                            zps = pz.tile([128, 1024], F32, tag="z")
                            for n0, nw in ((0, 512), (512, 258)):
                                for k in range(KH):
                                    nc.tensor.matmul(zps[:, n0:n0 + nw],
                                                     xts[:, k, 128 * m:128 * (m + 1)],
                                                     w1e[h][:, k, n0:n0 + nw],
                                                     start=(k == 0), stop=(k == KH - 1))
                            zb = feat.tile([128, H], F32R, tag="zw")
                            nc.scalar.copy(zb[:], zps[:, 0:H])
                            ec = colt[:, 16 * h + 2 * m:16 * h + 2 * m + 2]
                            nc.vector.tensor_copy(ec, zps[:, 768:770])
                            zsb.append(zb); elw_c.append(ec[:, 0:1]); erw_c.append(ec[:, 1:2])
                        zdsb, erd_c, eld_c = [], [], []
                        for m, rows in ((0, 128), (1, 127)):
                            zps = pz.tile([128, 1024], F32, tag="z")
                            for n0, nw in ((0, 512), (512, 258)):
                                nc.tensor.matmul(zps[0:rows, n0:n0 + nw],
                                                 eh[:, 128 * m:128 * m + rows],
                                                 ztab[h][:, n0:n0 + nw], start=True, stop=True)
                            zb = feat.tile([128, H], F32R, tag="zd")
                            nc.scalar.copy(zb[0:rows], zps[0:rows, 0:H])
                            ec = colt[:, 16 * h + 4 + 2 * m:16 * h + 4 + 2 * m + 2]
                            nc.vector.tensor_copy(ec[0:rows], zps[0:rows, 768:770])
                            zdsb.append(zb); eld_c.append(ec[0:rows, 0:1]); erd_c.append(ec[0:rows, 1:2])
                        vrow_transpose(pvec, SB, eld_c, [128, 127], eldb)
                        vrow_transpose(pvec, SB, elw_c, [128, 128], elwb)
                        alpW = attn.tile([128, 2, 256], F32R, tag="alw")
                        alpD = attn.tile([128, 2, 256], F32R, tag="ald")
                        alW = attn_block(attn, SB, erw_c, eldb, negw, [128, 128], S, alpW, True)
                        alD = attn_block(attn, SB, erd_c, elwb, negd, [128, 127], L, alpD, False)
                        aTW = alT_mm(palt, feat, alW, [128, 128], S, iwt, NP, "atw")
                        aTD = alT_mm(palt, feat, alD, [128, 127], L, idt, NP, "atd")
                        for wave in range(2):
                            pss = []
                            for mt in range(3 * wave, 3 * wave + 3):
                                ps = pagg.tile([128, NP], F32, tag="agg")
                                first = True
                                for ks, rows in ((0, 128), (1, 127)):
                                    nc.tensor.matmul(ps[:], zdsb[ks][0:rows, 128 * mt:128 * (mt + 1)],
                                                     aTW[ks][0:rows, :], start=first, stop=False)
                                    first = False
                                for ks in (0, 1):
                                    nc.tensor.matmul(ps[:], zsb[ks][:, 128 * mt:128 * (mt + 1)],
                                                     aTD[ks][:, :], start=False, stop=(ks == 1))
                                pss.append((mt, ps))
                            for mt, ps in pss:
                                ex = fet1.tile([128, N], F32, tag="elux")
                                nc.scalar.activation(ex[:], ps[:, 0:N], AF.Exp)
                                ts_(nc.vector, ex[:], ex[:], 1.0, -1.0, AL.min, AL.add)
                                nc.vector.scalar_tensor_tensor(
                                    h1T[:, KH * h + mt, :], ps[:, 0:N], 0.0, ex[:], op0=AL.max, op1=AL.add)
                    nc.sync.dma_start(h1spill[s], h1T[:])

        # ============ PHASE B1: layer-2 z + attention (own half) ============
        with tc.tile_pool(name="pB1", bufs=1) as pB1:
            w2e = pB1.tile([128, K2, 770], F32R)
            nc.sync.dma_start(w2e[:], W2[:])

            with tc.tile_pool(name="sampB", bufs=1) as sampB, \
                 tc.tile_pool(name="attnB", bufs=1) as attnB, \
                 tc.tile_pool(name="fetB", bufs=1) as fetB, \
                 tc.tile_pool(name="bigB", bufs=2) as bigB, \
                 tc.tile_pool(name="pzB", bufs=1, space="PSUM") as pzB, \
                 tc.tile_pool(name="pvecB", bufs=1, space="PSUM") as pvecB:

                for s in range(nsamp // 2):
                    h1T = bigB.tile([128, K2, N], F32R, tag="h1T")
                    nc.sync.dma_start(h1T[:], h1spill[s])
                    mskt = sampB.tile([128, 1024], BF16, tag="msk")
                    nc.sync.dma_start(mskt[:], msk_d[s])
                    negw = [mskt[:, 0:S], mskt[:, 256:256 + S]]
                    negd = [mskt[:, 512:768], mskt[:, 768:1024]]
                    SB = sampB.tile([128, 1344], F32, tag="scrb")
                    eldb, elwb = SB[:, 0:512], SB[:, 512:1024]

                    colt2 = fetB.tile([128, 16], F32, tag="cols2")
                    z2all = fetB.tile([128, 4, H], F32R, tag="z2all")
                    z2sb, er2_c, el2_c = [], [], []
                    for m, rows in ((0, 128), (1, 128), (2, 128), (3, 127)):
                        zps = pzB.tile([128, 1024], F32, tag="z")
                        for n0, nw in ((0, 512), (512, 258)):
                            for k in range(K2):
                                nc.tensor.matmul(zps[0:rows, n0:n0 + nw],
                                                 h1T[:, k, 128 * m:128 * m + rows],
                                                 w2e[:, k, n0:n0 + nw],
                                                 start=(k == 0), stop=(k == K2 - 1))
                        zb = z2all[:, m, :]
                        nc.scalar.copy(zb[0:rows], zps[0:rows, 0:H])
                        ec = colt2[:, 2 * m:2 * m + 2]
                        nc.vector.tensor_copy(ec[0:rows], zps[0:rows, 768:770])
                        z2sb.append(zb); el2_c.append(ec[0:rows, 0:1]); er2_c.append(ec[0:rows, 1:2])
                    vrow_transpose(pvecB, SB, el2_c[2:], [128, 127], eldb)
                    vrow_transpose(pvecB, SB, el2_c[:2], [128, 128], elwb)
                    alp2 = attnB.tile([128, 4, 256], F32R, tag="alp2")
                    attn_block(attnB, SB, er2_c[:2], eldb, negw, [128, 128], S, alp2[:, 0:2, :], True)
                    attn_block(attnB, SB, er2_c[2:], elwb, negd, [128, 127], L, alp2[:, 2:4, :], False)
                    nc.sync.dma_start(z2cc_in[s, :, 0:4 * H],
                                      z2all[:].rearrange("p c f -> p (c f)"))
                    nc.sync.dma_start(z2cc_in[s, :, 4 * H:4096],
                                      alp2[:].rearrange("p c f -> p (c f)"))

        # pairwise exchange of z2 + attention (the only cross-core dedup point)
        if PROFILE_NO_CC:
            nc.sync.dma_start(z2cc_out[0:nsamp // 2], z2cc_in[:])
            nc.sync.dma_start(z2cc_out[nsamp // 2:nsamp], z2cc_in[:])
        else:
            nc.gpsimd.collective_compute(
                "AllGather", AL.bypass,
                replica_groups=[[0, 1], [2, 3], [4, 5], [6, 7]],
                ins=[z2cc_in.ap().opt()], outs=[z2cc_out.ap().opt()])

        # ============ PHASE B2: alT + aggregation + xproj (all samples) =====
        with tc.tile_pool(name="pB2", bufs=1) as pB2:
            wihb = pB2.tile([128, KH + 1, G4], F32R)
            nc.sync.dma_start(wihb[:], Wihb[:])
            jwt = pB2.tile([128, 2, NP], F32R); nc.sync.dma_start(jwt[:], Jw[:])
            jdt = pB2.tile([128, 2, NP], F32R); nc.sync.dma_start(jdt[:], Jd[:])
            # bias rides in xproj slot nsamp (pad step zeroed host-side)
            nc.sync.dma_start(xproj[:, nsamp, :].unsqueeze(1), biasx[:].unsqueeze(1))

            with tc.tile_pool(name="sampC", bufs=2) as sampC, \
                 tc.tile_pool(name="featC", bufs=2) as featC, \
                 tc.tile_pool(name="bigC", bufs=1) as bigC, \
                 tc.tile_pool(name="paggC", bufs=3, space="PSUM") as paggC, \
                 tc.tile_pool(name="paltC", bufs=1, space="PSUM") as paltC:

                for s in range(nsamp):
                    pk = sampC.tile([128, 4096], F32R, tag="pk")
                    nc.sync.dma_start(pk[:], z2cc_out[s])
                    z2sb = [pk[:, H * m:H * (m + 1)] for m in range(4)]
                    alW2 = [pk[:, 3072:3328], pk[:, 3328:3584]]
                    alD2 = [pk[:, 3584:3840], pk[:, 3840:4096]]
                    aTW2 = alT_mm(paltC, featC, alW2, [128, 128], S, jwt, NP, "atw")
                    aTD2 = alT_mm(paltC, featC, alD2, [128, 127], L, jdt, NP, "atd")
                    gatT = bigC.tile([128, KH, NP], F32R, tag="gatT")
                    for wave in range(2):
                        pss = []
                        for mt in range(3 * wave, 3 * wave + 3):
                            ps = paggC.tile([128, NP], F32, tag="agg")
                            first = True
                            for ks, rows in ((0, 128), (1, 127)):
                                nc.tensor.matmul(ps[:], z2sb[2 + ks][0:rows, 128 * mt:128 * (mt + 1)],
                                                 aTW2[ks][0:rows, :], start=first, stop=False)
                                first = False
                            for ks in (0, 1):
                                nc.tensor.matmul(ps[:], z2sb[ks][:, 128 * mt:128 * (mt + 1)],
                                                 aTD2[ks][:, :], start=False, stop=(ks == 1))
                            pss.append((mt, ps))
                        for mt, ps in pss:
                            nc.scalar.copy(gatT[:, mt, :], ps[:])

                    # ---- xproj ----
                    for m in range(4):
                        pss = []
                        for ni in range(3):
                            ps = paggC.tile([128, 512], F32, tag="agg")
                            for k in range(KH):
                                nc.tensor.matmul(ps[:], gatT[:, k, 128 * m:128 * (m + 1)],
                                                 wihb[:, k, 512 * ni:512 * (ni + 1)],
                                                 start=(k == 0), stop=(k == KH - 1))
                            pss.append((ni, ps))
                        xsb = bigC.tile([128, G4], F32R, tag="xsb")
                        for ni, ps in pss:
                            nc.scalar.copy(xsb[:, 512 * ni:512 * (ni + 1)], ps[:])
                        nc.sync.dma_start(xproj[128 * m:128 * (m + 1), s:s + 1, :], xsb[:].unsqueeze(1))

        # ================= recurrence =================
        # gates packed host-side as [i|f|o|g]: one sigmoid over [0:3HH],
        # one tanh over [3HH:4HH].
        with tc.tile_pool(name="rx", bufs=2) as rx, \
             tc.tile_pool(name="rst", bufs=2) as rst, \
             tc.tile_pool(name="rg", bufs=2) as rg, \
             tc.tile_pool(name="pg", bufs=2, space="PSUM") as pgp, \
             tc.tile_pool(name="ptr", bufs=2, space="PSUM") as ptr, \
             tc.tile_pool(name="rfin", bufs=1) as rfin:
            whhr = rfin.tile([128, 3, G4], F32R)
            nc.sync.dma_start(whhr[:], Whh[:])
            W3 = 3 * nsamp
            hT = rst.tile([128, W3], F32R, tag="hT")
            zed = rfin.tile([128, W3], F32)
            nc.vector.memset(zed[:], 0.0)
            nc.vector.tensor_copy(hT[:], zed[:])
            cst = rst.tile([nsamp, HH], F32, tag="c")
            nc.vector.memset(cst[:], 0.0)
            snapA = rfin.tile([128, W3], F32)
            snapB = rfin.tile([128, W3], F32)
            SH3 = 3 * HH
            for t in range(nstep):
                tb = t % XBLK
                if tb == 0:
                    xrb = rx.tile([nsamp + 1, XBLK, G4], F32R, tag="xr")
                    nc.sync.dma_start(
                        xrb[:], xproj[t:t + XBLK].rearrange("t s g -> s t g"))
                ps = pgp.tile([nsamp, G4], F32, tag="gates")
                for b in range(3):
                    sl = slice(512 * b, 512 * (b + 1))
                    nc.tensor.matmul(ps[:, sl], i17[0:nsamp + 1, 0:nsamp], xrb[:, tb, sl],
                                     start=True, stop=False)
                    for kc in range(3):
                        nc.tensor.matmul(ps[:, sl], hT[:, nsamp * kc:nsamp * (kc + 1)],
                                         whhr[:, kc, sl], start=False, stop=(kc == 2))
                sg = rg.tile([nsamp, SH3], F32, tag="sg")
                nc.scalar.activation(sg[:], ps[:, 0:SH3], AF.Sigmoid)
                tg = rg.tile([nsamp, HH], F32, tag="tg")
                nc.scalar.activation(tg[:], ps[:, SH3:G4], AF.Tanh)
                t1 = rg.tile([nsamp, HH], F32, tag="t1")
                nc.vector.tensor_mul(t1[:], sg[:, HH:2 * HH], cst[:])
                t2 = rg.tile([nsamp, HH], F32, tag="t2")
                nc.vector.tensor_mul(t2[:], sg[:, 0:HH], tg[:])
                cn = rst.tile([nsamp, HH], F32, tag="c")
                nc.vector.tensor_add(cn[:], t1[:], t2[:])
                th = rg.tile([nsamp, HH], F32, tag="th")
                nc.scalar.activation(th[:], cn[:], AF.Tanh)
                hh = rg.tile([nsamp, HH], F32, tag="hh")
                nc.vector.tensor_mul(hh[:], sg[:, 2 * HH:SH3], th[:])
                tps = ptr.tile([128, W3], F32, tag="tr")
                for kc in range(3):
                    nc.tensor.matmul(tps[:, nsamp * kc:nsamp * (kc + 1)],
                                     hh[:, 128 * kc:128 * (kc + 1)], ident[0:nsamp, 0:nsamp],
                                     is_transpose=True, start=True, stop=True)
                hTn = rst.tile([128, W3], F32R, tag="hT")
                nc.vector.tensor_copy(hTn[:], tps[:])
                hT = hTn
                cst = cn
                if t == nstep - 2:
                    nc.vector.tensor_copy(snapA[:], tps[:])
                if t == nstep - 1:
                    nc.vector.tensor_copy(snapB[:], tps[:])
            hsel = rfin.tile([128, W3], F32)
            ts_(nc.vector, hsel[:], snapA[:], flg[:, 0:1], None, AL.mult)
            nc.vector.scalar_tensor_tensor(hsel[:], snapB[:], flg[:, 1:2], hsel[:],
                                           op0=AL.mult, op1=AL.add)
            nc.sync.dma_start(cc_in[:].rearrange("(c p) f -> p c f", p=128),
                              hsel[:].rearrange("p (c f) -> p c f", c=3))

        # ================= collective + bilinear =================
        with tc.tile_pool(name="bil", bufs=1) as bil, \
             tc.tile_pool(name="pbil", bufs=1, space="PSUM") as pbil:
            if PROFILE_NO_CC:
                nc.sync.dma_start(cc_out[0:3 * 128, :], cc_in[:])
            else:
                nc.gpsimd.collective_compute(
                    "AllGather", AL.bypass,
                    replica_groups=[[0, 1, 2, 3], [4, 5, 6, 7]],
                    ins=[cc_in.ap().opt()], outs=[cc_out.ap().opt()])
            gath = bil.tile([128, 12, nsamp], F32)
            nc.sync.dma_start(gath[:], cc_out[:].rearrange("(c p) f -> p c f", p=128))
            pT = bil.tile([128, KH, nsamp], F32R)
            nc.vector.tensor_copy(pT[:], gath[:, 0:KH, :])
            hps = pbil.tile([nsamp, H], F32, tag="htr")
            for kc in range(KH):
                nc.tensor.matmul(hps[:, 128 * kc:128 * (kc + 1)],
                                 gath[:, KH + kc, :], ident[:],
                                 is_transpose=True, start=True, stop=True)
            hsb = bil.tile([nsamp, H], F32)
            nc.scalar.copy(hsb[:], hps[:])
            bwsb = bil.tile([128, KH, H], F32)
            bwr = bil.tile([128, KH, H], F32R)
            outc = bil.tile([nsamp, NL], F32)
            junk = bil.tile([nsamp, H], F32)
            for k in range(NL):
                nc.sync.dma_start(bwsb[:], bilW[k])
                for c in range(KH):
                    nc.vector.tensor_copy(bwr[:, c, :], bwsb[:, c, :])
                vps = pbil.tile([nsamp, H], F32, tag="v")
                for n0 in (0, 512):
                    nw = min(512, H - n0)
                    for kc in range(KH):
                        nc.tensor.matmul(vps[:, n0:n0 + nw], pT[:, kc, :],
                                         bwr[:, kc, n0:n0 + nw],
                                         start=(kc == 0), stop=(kc == KH - 1))
                nc.vector.scalar_tensor_tensor(junk[:], vps[:], 1.0, hsb[:],
                                               op0=AL.mult, op1=AL.mult,
                                               accum_out=outc[:, k:k + 1])
            bbt = bil.tile([nsamp, NL], F32)
            nc.sync.dma_start(bbt[:], bilb[:])
            outt = bil.tile([nsamp, NL], F32)
            nc.vector.tensor_add(outt[:], outc[:], bbt[:])
            nc.sync.dma_start(out[:], outt[:])


# ===================== host-side preparation =====================
def _chunkP(a):
    """[X*128, ...] -> [128, X, ...] with p inner: out[p, c, ...] = a[128c+p]"""
    x = a.reshape(a.shape[0] // 128, 128, *a.shape[1:])
    return np.swapaxes(x, 0, 1)

def _bf(a):
    return np.ascontiguousarray(a.astype(ml_dtypes.bfloat16))

def _f(a):
    return np.ascontiguousarray(np.asarray(a, np.float32))

# gate reorder: torch [i|f|g|o] -> kernel [i|f|o|g]
_GPERM = np.concatenate([np.arange(0, HH), np.arange(HH, 2 * HH),
                         np.arange(3 * HH, 4 * HH), np.arange(2 * HH, 3 * HH)])


def _build_masks(spans, nsamp):
    """spans [nsamp,S,3] int -> (msk [nsamp,128,1024] bf16, eh [nsamp,DEP,S] f32).

    msk cols: 0:255 negw(nodes 0..127) | 256:511 negw(nodes 128..255)
            | 512:768 negd(deps 0..127) | 768:1024 negd(deps 128..254)
    """
    sp = np.asarray(spans)
    w0, w1, lab = sp[:, :, 0], sp[:, :, 1], sp[:, :, 2]   # [nsamp, S]
    nodes = np.arange(256)
    adjW = ((w0[:, None, :] == nodes[None, :, None]) |
            (w1[:, None, :] == nodes[None, :, None]))     # [nsamp,256,S]
    negW = (adjW.astype(np.float32) - 1.0) * 1e9
    words = np.arange(256)
    adjD = ((w0[:, :, None] == words[None, None, :]) |
            (w1[:, :, None] == words[None, None, :]))     # [nsamp,S,256]
    negD = (adjD.astype(np.float32) - 1.0) * 1e9
    msk = np.full((nsamp, 128, 1024), -1e9, np.float32)
    msk[:, :, 0:S] = negW[:, 0:128]
    msk[:, :, 256:256 + S] = negW[:, 128:256]
    msk[:, :, 512:768] = negD[:, 0:128]
    msk[:, 0:127, 768:1024] = negD[:, 128:255]
    eh = (lab[:, None, :] == np.arange(DEP)[None, :, None]).astype(np.float32)
    return _bf(msk), _f(eh)


def make_in_maps(inp, nsamp=16):
    B = np.asarray(inp["prem_hidden_states"]).shape[0]
    emb = _f(inp["depend_emb"])
    in_maps = []
    onehot = lambda idx, w: np.eye(w, dtype=np.float32)[idx]  # rows
    # placement mats (constant)
    wid = np.arange(256)
    Iw_ = _f(_chunkP(onehot(wid, NP)))                # -> [128,2,NP]
    sid = np.arange(255)
    Id_rows = np.zeros((256, NP), np.float32); Id_rows[:255] = onehot(256 + sid, NP)
    Id_ = _f(_chunkP(Id_rows))
    ident_ = np.eye(128, dtype=np.float32)
    for c in range(8):
        unit, half = c % 4, c // 4
        g = "prem" if unit < 2 else "hypo"
        fwd = (unit % 2 == 0)
        lstm = "lstm1" if unit < 2 else "lstm2"
        dirn = "f" if fwd else "b"
        sl = slice(16 * half, 16 * half + nsamp)
        hid = _f(inp[f"{g}_hidden_states"])[sl]       # [nsamp, L, H]
        spans = np.asarray(inp[f"{g}_span"])[sl]      # [nsamp, S, 3]
        # pair dedup: fwd cores own local samples 0..7, bwd cores 8..15
        own = slice(0, nsamp // 2) if fwd else slice(nsamp // 2, nsamp)
        hid_o, spans_o = hid[own], spans[own]
        m = {}
        m["xT"] = _f(np.stack([_chunkP(hid_o[s].T) for s in range(nsamp // 2)]))
        m["msk"], m["ehm"] = _build_masks(spans_o, nsamp // 2)
        m["identity"] = ident_
        W1 = _f(inp[f"{g}_W1"])                       # [2,H,H]
        a1 = _f(inp[f"{g}_a1"])                       # [2, 2H]
        # extended weights: cols 768+lr hold W @ a_half (attention vectors)
        W1x = np.zeros((2, H, 770), np.float32)
        W1x[:, :, 0:H] = W1
        for h in range(2):
            for lr in range(2):
                W1x[h, :, 768 + lr] = W1[h] @ a1[h, lr * H:(lr + 1) * H]
        m["W1"] = _f(np.stack([_chunkP(W1x[h]) for h in range(2)]))
        m["ztab_in"] = _f(np.stack([emb @ W1x[h] for h in range(2)]))
        W2 = _f(inp[f"{g}_W2"])                       # [2H, H]
        a2 = _f(inp[f"{g}_a2"])                       # [2H]
        W2x = np.zeros((2 * H, 770), np.float32)
        W2x[:, 0:H] = W2
        for lr in range(2):
            W2x[:, 768 + lr] = W2 @ a2[lr * H:(lr + 1) * H]
        m["W2"] = _f(_chunkP(W2x))
        m["Iw"] = Iw_; m["Id"] = Id_
        if fwd:
            Jw_r = onehot(wid, NP)
            Jd_rows = np.zeros((256, NP), np.float32); Jd_rows[:255] = onehot(256 + sid, NP)
            ones_ = np.ones((1, NP), np.float32); ones_[0, N] = 0.0
        else:
            Jw_r = onehot(511 - wid, NP)
            Jd_rows = np.zeros((256, NP), np.float32); Jd_rows[:255] = onehot(255 - sid, NP)
            ones_ = np.ones((1, NP), np.float32); ones_[0, 0] = 0.0
        m["Jw"] = _f(_chunkP(Jw_r)); m["Jd"] = _f(_chunkP(Jd_rows))
        m["ones"] = _f(ones_)
        Wih = _f(inp[f"{lstm}_Wih_{dirn}"])           # [4HH, H]
        bb = _f(inp[f"{lstm}_b_{dirn}"])              # [4HH]
        Wihb_ = np.zeros((896, G4), np.float32)
        Wihb_[:H] = Wih.T[:, _GPERM]
        Wihb_[H] = bb[_GPERM]
        m["Wihb"] = _f(_chunkP(Wihb_))                # [128, 7, G4]
        biasx_ = np.tile(bb[_GPERM][None, :], (NP, 1)).astype(np.float32)
        biasx_[N if fwd else 0] = 0.0
        m["biasx"] = _f(biasx_)
        Whh_ = _f(inp[f"{lstm}_Whh_{dirn}"])          # [4HH, HH]
        m["Whh"] = _f(_chunkP(Whh_.T[:, _GPERM]))     # [128, 3, G4]
        fl = np.zeros((128, 2), np.float32)
        fl[:, 0] = 1.0 if fwd else 0.0
        fl[:, 1] = 0.0 if fwd else 1.0
        m["flags"] = fl
        bilW = _f(inp["bil_W"])                       # [3,H,H]
        m["bilW"] = _f(np.stack([_chunkP(bilW[k]) for k in range(NL)]))
        m["bilb"] = _f(np.broadcast_to(_f(inp["bil_b"])[None, :], (nsamp, NL)).copy())
        in_maps.append(m)
    return in_maps


# ===================== harness entry point =====================
_NC_CACHE = {}

def _get_nc(nsamp=16, nstep=NP):
    key = (nsamp, nstep)
    if key not in _NC_CACHE:
        _NC_CACHE[key] = build_nc(nsamp=nsamp, nstep=nstep)
    return _NC_CACHE[key]


def kernel(**inputs):
    """Full-input entry: shards across 8 NeuronCores, runs the Bass kernel,
    returns the full [32, 3] float32 output."""
    inputs = {k: np.asarray(v) for k, v in inputs.items()}
    nc = _get_nc()
    in_maps = make_in_maps(inputs, nsamp=16)
    from concourse import bass_utils
    res = bass_utils.run_bass_kernel_spmd(nc, in_maps, core_ids=list(range(8)))
    out = np.concatenate([res.results[0]["out"], res.results[4]["out"]], 0)
    return out.astype(np.float32)

